# revision 9
# baseline (speedup 1.0000x reference)
"""Two-layer GCN + MLP on 8 Trainium2 NeuronCores — dense-A formulation.

Math: with A-hat = D^-1/2 (A + I) D^-1/2 and dinv = deg^-1/2,
  h  = relu((A-hat @ x) @ W1 + b1)        (aggregation commutes with W1)
  g  = A-hat @ (h @ W2) + b2
  out = relu(g @ Wm1 + bm1) @ Wm2 + bm2
A-hat @ v = dinv * ((A+I) @ (dinv * v)).  The edge multiplicity matrix
(A+I) holds small integers — EXACT in fp8e4m3 — so each aggregation is a
dense fp8 DoubleRow matmul over 80 source chunks of 128 rows against the
host-precomputed y = dinv*x table (fp8, the only quantization).  The
trailing dinv row scale stays exact via the f32 PSUM->SBUF activation
copy.  b2 folds into the MLP bias (bm1_eff = b2 @ Wm1 + bm1).

Sharding: nodes (dst columns of A) split across 8 cores; each core holds
A^T [10240 src, 1280 dst] fp8 = 13.1 MB, SBUF-resident, reused by BOTH
layers.  Everything stays in GLOBAL source-chunk order so the program is
core-independent; per-core differences live only in the input data.
Layer 2 needs t2 = dinv*(h@W2) from all cores: exchanged in fp8 via two
AllGathers over dst-tile halves [0..4] / [5..9]; each core's own block
echoes back through the collective.  The source-chunk dim is host-
permuted to [half0-chunks | half1-chunks] so tbuf loads are single
3D-AP DMAs and the layer-2 phases are contiguous pair ranges.

Layer-1 aggregation streams slice-wise with the A DMA (5 concurrent PSUM
bank accumulators over dst tiles 0-4, then tiles 5-9 from SBUF).  DMA
count is minimized (~11: per-DMA fixed cost dominates on this fabric):
weights packed into 2 inputs, t2own/out stores staged in SBUF, loads on
the SP queue, stores + tbuf loads on the Activation queue.
"""
import sys
sys.path.insert(0, "/opt/trn_rl_repo")

import numpy as np
import ml_dtypes

F8NP = ml_dtypes.float8_e4m3
BF16 = ml_dtypes.bfloat16

N, D, H = 10000, 256, 512
NC = 8
NSH = N // NC          # 1250 nodes per core
P = 128
NT = 10                # dst tiles per core
NPAD = NT * P          # 1280 padded rows per core
SC = NC * NT           # 80 global src chunks
NPR = SC // 2          # 40 chunk pairs
T0 = 5                 # dst tiles in exchange half 0
H0R = T0 * P           # 640 rows in half 0
H1R = NPAD - H0R       # 640 rows in half 1
SLC = 20               # src chunks per A-load slice
# global src-chunk permutation: exchange-half-0 chunks (each core's dst
# tiles 0..T0-1) first, then half-1 chunks -- makes both tbuf->SBUF loads
# a single contiguous 3D-AP DMA and both layer-2 phases contiguous pair
# ranges.  Applied host-side to A and y chunk dims (core-independent).
PERM = ([NT * j + c for j in range(NC) for c in range(T0)]
        + [NT * j + T0 + c for j in range(NC) for c in range(NT - T0)])
NCH0 = NC * T0         # 40 half-0 chunks
PAIRS_H0 = list(range(NCH0 // 2))
PAIRS_H1 = list(range(NCH0 // 2, NPR))
# packed bf16 weight offsets (cols)
OW1, OW2, OM1, OM2, OB2 = 0, 1024, 2048, 3072, 4096
WBF_COLS = 4352
# packed f32 offsets
OB1, OBM, ODI = 0, 4, 8
WF_COLS = 18

_cache = {}
_last_res = None
_last_in_maps = None


def _build():
    from concourse import bacc, tile, mybir
    from concourse.masks import make_identity

    f32 = mybir.dt.float32
    bf16 = mybir.dt.bfloat16
    f8 = mybir.dt.float8e4
    DR = mybir.MatmulPerfMode.DoubleRow
    ACT = mybir.ActivationFunctionType

    nc = bacc.Bacc("TRN2", target_bir_lowering=False, debug=False,
                   enable_asserts=True, num_devices=NC)

    A_d = nc.dram_tensor("Ashard", [P, SC, NPAD], f8, kind="ExternalInput").ap()
    y_d = nc.dram_tensor("ytabs", [P, SC, D], f8, kind="ExternalInput").ap()
    wbf_d = nc.dram_tensor("wbf", [P, WBF_COLS], bf16, kind="ExternalInput").ap()
    wf_d = nc.dram_tensor("wf", [P, WF_COLS], f32, kind="ExternalInput").ap()
    out_d = nc.dram_tensor("out", [NPAD, D], bf16, kind="ExternalOutput").ap()

    out_r = out_d.rearrange("(t p) d -> p t d", p=P)

    with tile.TileContext(nc) as tc:
        with tc.tile_pool(name="cst", bufs=1) as cst, \
             tc.tile_pool(name="big", bufs=1) as big, \
             tc.tile_pool(name="work", bufs=3) as work, \
             tc.tile_pool(name="pz", bufs=5, space="PSUM") as pz, \
             tc.tile_pool(name="ptr", bufs=1, space="PSUM") as ptr, \
             tc.tile_pool(name="ph", bufs=1, space="PSUM") as ph, \
             tc.tile_pool(name="pt", bufs=1, space="PSUM") as pt, \
             tc.tile_pool(name="dram", bufs=1, space="DRAM") as dram:

            A_sb = big.tile([P, SC, NPAD], f8, name="A_sb")
            y_sb = big.tile([P, SC, D], f8, name="y_sb")
            t2_sb = big.tile([P, SC, D], f8, name="t2_sb")
            t2st = big.tile([P, NT, D], f8, name="t2st")
            ost = big.tile([P, NT, D], bf16, name="ost")
            pa2 = big.tile([P, NT, D], f32, name="pa2")

            wf_t = cst.tile([P, WF_COLS], f32)
            wbf_t = cst.tile([P, WBF_COLS], bf16)
            # SP queue: consts, y, A slices streaming
            nc.sync.dma_start(out=wf_t[:], in_=wf_d[:])
            nc.sync.dma_start(out=y_sb[:], in_=y_d[:])
            nc.sync.dma_start(out=wbf_t[:], in_=wbf_d[:])
            for s in range(SC // SLC):
                nc.sync.dma_start(out=A_sb[:, s * SLC:(s + 1) * SLC, :],
                                  in_=A_d[:, s * SLC:(s + 1) * SLC, :])

            ident_t = cst.tile([P, P], bf16)
            make_identity(nc, ident_t[:])
            ones_b = cst.tile([1, P], bf16)
            nc.any.memset(ones_b[:], 1.0)

            dinv = wf_t[:, ODI:ODI + NT]

            t2own = dram.tile([NPAD, D], f8, name="t2own")
            t2own_r = t2own[:].rearrange("(t p) d -> p t d", p=P)
            tbuf0 = dram.tile([NC * H0R, D], f8, addr_space="Shared",
                              name="tbuf0")
            tbuf1 = dram.tile([NC * H1R, D], f8, addr_space="Shared",
                              name="tbuf1")

            def agg_mm(z, pr, t, rhs_sb, start, stop):
                nc.tensor.matmul(
                    out=z[:], lhsT=A_sb[:, 2 * pr:2 * pr + 2, t * P:(t + 1) * P],
                    rhs=rhs_sb[:, 2 * pr:2 * pr + 2, :],
                    perf_mode=DR, start=start, stop=stop)

            def transpose_in(dst, src_ap, nblk):
                for j in range(nblk):
                    ps = ptr.tile([P, P], bf16, name="ps_tr")
                    nc.tensor.transpose(out=ps[:], in_=src_ap[:, j * P:(j + 1) * P],
                                        identity=ident_t[:])
                    nc.vector.tensor_copy(out=dst[:, j, :], in_=ps[:])

            def dense_T(psum_hT, w_off, xT, b_off, act_out):
                for hc in range(4):
                    for j in (0, 1):
                        nc.tensor.matmul(
                            out=psum_hT[:, hc, :],
                            lhsT=wbf_t[:, w_off + j * 512 + hc * P:
                                       w_off + j * 512 + (hc + 1) * P],
                            rhs=xT[:, j, :], start=(j == 0), stop=(j == 1))
                    nc.scalar.activation(out=act_out[:, hc, :],
                                         in_=psum_hT[:, hc, :],
                                         func=ACT.Relu,
                                         bias=wf_t[:, b_off + hc:b_off + hc + 1])

            def dense_tile(t, z):
                """dinv-scale agg, W1+relu, W2, dinv-scale -> t2st."""
                agg1 = work.tile([P, D], bf16, name="agg1")
                nc.scalar.activation(out=agg1[:], in_=z[:], func=ACT.Copy,
                                     scale=dinv[:, t:t + 1])
                aggT = work.tile([P, 2, P], bf16, name="aggT")
                transpose_in(aggT, agg1, 2)
                psum_hT = ph.tile([P, 4, P], f32, name="psum_h")
                h_sbT = work.tile([P, 4, P], bf16, name="h_sbT")
                dense_T(psum_hT, OW1, aggT, OB1, h_sbT)
                psum_t = pt.tile([P, D], f32, name="psum_t")
                for j in range(4):
                    nc.tensor.matmul(
                        out=psum_t[:], lhsT=h_sbT[:, j, :],
                        rhs=wbf_t[:, OW2 + j * D:OW2 + (j + 1) * D],
                        start=(j == 0), stop=(j == 3))
                nc.scalar.activation(out=t2st[:, t, :], in_=psum_t[:],
                                     func=ACT.Copy, scale=dinv[:, t:t + 1])

            # ---- layer 1, wave A (tiles 0..4): slice-major streaming
            WA = 5
            zs = [pz.tile([P, D], f32, name="z") for _ in range(WA)]
            for s in range(SC // SLC):
                for t in range(WA):
                    for pr in range(s * SLC // 2, (s + 1) * SLC // 2):
                        agg_mm(zs[t], pr, t, y_sb,
                               start=(pr == 0), stop=(pr == NPR - 1))
            for t in range(WA):
                dense_tile(t, zs[t])
                if t == T0 - 1:
                    nc.scalar.dma_start(out=t2own_r[:, 0:T0, :],
                                        in_=t2st[:, 0:T0, :])
                    nc.gpsimd.collective_compute(
                        "AllGather", mybir.AluOpType.bypass,
                        replica_groups=[list(range(NC))],
                        ins=[t2own[0:H0R, :].opt()], outs=[tbuf0[:].opt()],
                    )

            # ---- layer 1, wave B (tiles 5..9): A fully resident
            for t in range(WA, NT):
                z = pz.tile([P, D], f32, name="z")
                for pr in range(NPR):
                    agg_mm(z, pr, t, y_sb, start=(pr == 0), stop=(pr == NPR - 1))
                dense_tile(t, z)
            nc.scalar.dma_start(out=t2own_r[:, T0:NT, :], in_=t2st[:, T0:NT, :])
            with tc.high_priority():
                nc.gpsimd.collective_compute(
                    "AllGather", mybir.AluOpType.bypass,
                    replica_groups=[list(range(NC))],
                    ins=[t2own[H0R:NPAD, :].opt()], outs=[tbuf1[:].opt()],
                )

            # ---- exchange half 0 -> t2_sb chunks [0:NCH0], one DMA
            nc.scalar.dma_start(
                out=t2_sb[:, 0:NCH0, :],
                in_=tbuf0[:].rearrange("(c p) d -> p c d", p=P))

            # ---- layer 2 phase H0
            for t in range(NT):
                z = pz.tile([P, D], f32, name="z")
                for i, pr in enumerate(PAIRS_H0):
                    agg_mm(z, pr, t, t2_sb, start=(i == 0),
                           stop=(i == len(PAIRS_H0) - 1))
                nc.scalar.activation(out=pa2[:, t, :], in_=z[:], func=ACT.Copy,
                                     scale=dinv[:, t:t + 1])

            # ---- exchange half 1 -> t2_sb chunks [NCH0:80], one DMA
            nc.scalar.dma_start(
                out=t2_sb[:, NCH0:SC, :],
                in_=tbuf1[:].rearrange("(c p) d -> p c d", p=P))

            # ---- layer 2 phase H1 + MLP per tile
            for t in range(NT):
                z = pz.tile([P, D], f32, name="z")
                for i, pr in enumerate(PAIRS_H1):
                    agg_mm(z, pr, t, t2_sb, start=(i == 0),
                           stop=(i == len(PAIRS_H1) - 1))
                gagg = work.tile([P, D], bf16, name="gagg")
                nc.vector.scalar_tensor_tensor(
                    out=gagg[:], in0=z[:], scalar=dinv[:, t:t + 1],
                    in1=pa2[:, t, :], op0=mybir.AluOpType.mult,
                    op1=mybir.AluOpType.add)
                gT = work.tile([P, 2, P], bf16, name="gT")
                transpose_in(gT, gagg, 2)
                psum_oT = ph.tile([P, 4, P], f32, name="psum_h")
                o1T = work.tile([P, 4, P], bf16, name="o1T")
                dense_T(psum_oT, OM1, gT, OBM, o1T)
                psum_y = pt.tile([P, D], f32, name="psum_t")
                for j in range(4):
                    nc.tensor.matmul(
                        out=psum_y[:], lhsT=o1T[:, j, :],
                        rhs=wbf_t[:, OM2 + j * D:OM2 + (j + 1) * D],
                        start=(j == 0), stop=False)
                nc.tensor.matmul(out=psum_y[:], lhsT=ones_b[:1, :],
                                 rhs=wbf_t[0:1, OB2:OB2 + D],
                                 start=False, stop=True)
                nc.scalar.activation(out=ost[:, t, :], in_=psum_y[:],
                                     func=ACT.Copy)
            nc.scalar.dma_start(out=out_r[:], in_=ost[:])

    nc.finalize()
    return nc


def _make_in_maps(x, edge_index, W1, b1, W2, b2, Wm1, bm1, Wm2, bm2):
    x = np.asarray(x, dtype=np.float32)
    src = np.asarray(edge_index[0], dtype=np.int64)
    dst = np.asarray(edge_index[1], dtype=np.int64)
    deg = 1 + np.bincount(dst, minlength=N).astype(np.int64)
    dinv = (1.0 / np.sqrt(deg.astype(np.float32))).astype(np.float32)

    # dense multiplicity matrix (A + I) in padded global coordinates;
    # small-int entries are exact in fp8e4m3 (bit patterns via LUT)
    srcp = (src // NSH) * NPAD + (src % NSH)
    dstp = (dst // NSH) * NPAD + (dst % NSH)
    GP = NC * NPAD
    mcnt = np.zeros((GP, GP), np.uint8)
    np.add.at(mcnt, (srcp, dstp), 1)
    rp = (np.arange(N) // NSH) * NPAD + (np.arange(N) % NSH)
    mcnt[rp, rp] += 1
    lut = np.arange(256, dtype=np.float32).astype(F8NP).view(np.uint8)
    abits = lut[mcnt]

    yf = np.zeros((NC, NPAD, D), np.float32)
    dinvf = np.zeros((NC, NPAD), np.float32)
    for k in range(NC):
        yf[k, :NSH] = x[k * NSH:(k + 1) * NSH] * dinv[k * NSH:(k + 1) * NSH, None]
        dinvf[k, :NSH] = dinv[k * NSH:(k + 1) * NSH]
    y8 = yf.reshape(GP, D).astype(F8NP)
    ytabs = np.ascontiguousarray(
        y8.reshape(SC, P, D)[PERM].transpose(1, 0, 2))

    bf16 = BF16
    wbf = np.zeros((P, WBF_COLS), bf16)
    wbf[:, OW1:OW1 + 1024] = np.asarray(W1, np.float32).reshape(
        2, P, H).transpose(1, 0, 2).reshape(P, 1024).astype(bf16)
    wbf[:, OW2:OW2 + 1024] = np.asarray(W2, np.float32).reshape(
        4, P, D).transpose(1, 0, 2).reshape(P, 1024).astype(bf16)
    wbf[:, OM1:OM1 + 1024] = np.asarray(Wm1, np.float32).reshape(
        2, P, H).transpose(1, 0, 2).reshape(P, 1024).astype(bf16)
    wbf[:, OM2:OM2 + 1024] = np.asarray(Wm2, np.float32).reshape(
        4, P, D).transpose(1, 0, 2).reshape(P, 1024).astype(bf16)
    wbf[:, OB2:OB2 + D] = np.asarray(bm2, np.float32).reshape(1, D).astype(bf16)

    bm1e = (np.asarray(b2, np.float32) @ np.asarray(Wm1, np.float32)
            + np.asarray(bm1, np.float32))

    in_maps = []
    for k in range(NC):
        ashard = np.ascontiguousarray(
            abits[:, k * NPAD:(k + 1) * NPAD].reshape(SC, P, NPAD)[PERM]
            .transpose(1, 0, 2)).view(F8NP)
        wf = np.zeros((P, WF_COLS), np.float32)
        wf[:, OB1:OB1 + 4] = np.asarray(b1, np.float32).reshape(4, P).T
        wf[:, OBM:OBM + 4] = bm1e.reshape(4, P).T
        wf[:, ODI:ODI + NT] = dinvf[k].reshape(NT, P).T
        in_maps.append(dict(Ashard=ashard, ytabs=ytabs, wbf=wbf, wf=wf))
    return in_maps, ()


def kernel(x, edge_index, W1, b1, W2, b2, Wm1, bm1, Wm2, bm2):
    from concourse.bass_utils import run_bass_kernel_spmd

    in_maps, key = _make_in_maps(x, edge_index, W1, b1, W2, b2,
                                 Wm1, bm1, Wm2, bm2)
    if key not in _cache:
        _cache[key] = _build()
    nc = _cache[key]

    global _last_res, _last_in_maps
    _last_in_maps = in_maps
    res = run_bass_kernel_spmd(nc, in_maps, core_ids=list(range(NC)))
    _last_res = res
    out = np.concatenate(
        [res.results[k]["out"][:NSH] for k in range(NC)], axis=0)
    return out.astype(np.float32)
